# revision 2
# baseline (speedup 1.0000x reference)
"""Trainium2 kernel for nn_DSLRCollisionDecoder.

Data-parallel over batch B=256 across 8 NeuronCores (32 examples/core).
Device computes the dominant work: the pairwise 48->64->64->64 gelu MLP
with skip connection over B*K*K = 262144 pairs, packed 2 pairs/column
via block-diagonal weights so matmul/ACT run at full 128-partition width.
bf16 weights/activations/IO (fp32 PSUM accumulate) give 1-cycle/row
matmuls and half the DMA bytes; gelu runs on ScalarE over 2048-column
PSUM spans to amortize instruction overhead.
Host does index gathers, the small per-pair geometry (rotation frames),
and final channel concat.
"""
import sys
import numpy as np
from scipy.special import erf

sys.path.insert(0, "/opt/trn_rl_repo")

B, N, K = 256, 64, 32
EPS = 1e-8
NCORES = 8
BPC = B // NCORES          # batches per core
PAIRS = BPC * K * K        # 32768 pairs per core
NCOL = PAIRS // 2          # 16384 columns (2 pairs per column)
MTILE = 2048               # macro-tile columns (4 PSUM banks)
NMT = NCOL // MTILE        # 8 macro tiles
MM = 512                   # matmul free dim (1 PSUM bank)

_prog_cache = {}


def _gelu_np(x):
    return 0.5 * x * (1.0 + erf(x / np.sqrt(2.0).astype(np.float32)))


def _build_program():
    if "nc" in _prog_cache:
        return _prog_cache["nc"]
    import concourse.bacc as bacc
    import concourse.tile as tile
    from concourse import mybir
    from concourse.alu_op_type import AluOpType
    from bass_rust import ActivationFunctionType as AF

    F32 = mybir.dt.float32
    BF16 = mybir.dt.bfloat16
    nc = bacc.Bacc("TRN2", target_bir_lowering=False, debug=False,
                   num_devices=NCORES)
    ft_d = nc.declare_dram_parameter("featT", [96, NCOL], BF16, isOutput=False)
    w1_d = nc.declare_dram_parameter("w1bd", [96, 128], BF16, isOutput=False)
    w2_d = nc.declare_dram_parameter("w2bd", [128, 128], BF16, isOutput=False)
    w3_d = nc.declare_dram_parameter("w3bd", [128, 128], BF16, isOutput=False)
    b1_d = nc.declare_dram_parameter("b1bd", [128, 1], F32, isOutput=False)
    b2_d = nc.declare_dram_parameter("b2bd", [128, 1], F32, isOutput=False)
    b3_d = nc.declare_dram_parameter("b3bd", [128, 1], F32, isOutput=False)
    out_d = nc.declare_dram_parameter("embT", [128, NCOL], BF16, isOutput=True)

    with tile.TileContext(nc) as tc:
        with (
            tc.tile_pool(name="w", bufs=1) as wp,
            tc.tile_pool(name="io", bufs=3) as iop,
            tc.tile_pool(name="act", bufs=2) as ac,
            tc.tile_pool(name="ps", bufs=2, space="PSUM") as pp,
        ):
            tw1 = wp.tile([96, 128], BF16, tag="w1")
            tw2 = wp.tile([128, 128], BF16, tag="w2")
            tw3 = wp.tile([128, 128], BF16, tag="w3")
            tb1 = wp.tile([128, 1], F32, tag="b1")
            tb2 = wp.tile([128, 1], F32, tag="b2")
            tb3 = wp.tile([128, 1], F32, tag="b3")
            nc.sync.dma_start(tw1[:], w1_d[:, :])
            nc.sync.dma_start(tw2[:], w2_d[:, :])
            nc.sync.dma_start(tw3[:], w3_d[:, :])
            nc.sync.dma_start(tb1[:], b1_d[:, :])
            nc.sync.dma_start(tb2[:], b2_d[:, :])
            nc.sync.dma_start(tb3[:], b3_d[:, :])
            for i in range(NMT):
                sl = slice(i * MTILE, (i + 1) * MTILE)
                ft = iop.tile([96, MTILE], BF16, tag="ft")
                nc.sync.dma_start(ft[:], ft_d[:, sl])
                ps1 = pp.tile([128, MTILE], F32, tag="ps")
                for j in range(MTILE // MM):
                    jm = slice(j * MM, (j + 1) * MM)
                    nc.tensor.matmul(ps1[:, jm], tw1[:], ft[:, jm],
                                     start=True, stop=True)
                x1 = ac.tile([128, MTILE], BF16, tag="x1")
                nc.scalar.activation(x1[:], ps1[:], AF.Gelu, bias=tb1[:, :])
                ps2 = pp.tile([128, MTILE], F32, tag="ps")
                for j in range(MTILE // MM):
                    jm = slice(j * MM, (j + 1) * MM)
                    nc.tensor.matmul(ps2[:, jm], tw2[:], x1[:, jm],
                                     start=True, stop=True)
                x2 = ac.tile([128, MTILE], BF16, tag="x2")
                nc.scalar.activation(x2[:], ps2[:], AF.Gelu, bias=tb2[:, :])
                ps3 = pp.tile([128, MTILE], F32, tag="ps")
                for j in range(MTILE // MM):
                    jm = slice(j * MM, (j + 1) * MM)
                    nc.tensor.matmul(ps3[:, jm], tw3[:], x2[:, jm],
                                     start=True, stop=True)
                x3 = ac.tile([128, MTILE], BF16, tag="x3")
                nc.scalar.activation(x3[:], ps3[:], AF.Gelu, bias=tb3[:, :])
                emb = ac.tile([128, MTILE], BF16, tag="emb")
                nc.vector.tensor_tensor(emb[:], x3[:], x1[:], op=AluOpType.add)
                nc.sync.dma_start(out_d[:, sl], emb[:])
    nc.compile()
    _prog_cache["nc"] = nc
    return nc


def _geometry(z_a, z_b, fps_a, fps_b, a_idx, b_idx,
              pos_w1, pos_b1, pos_w2, pos_b2):
    """Gathers + per-pair frame/rotation/pos-MLP; returns feat + concat parts."""
    zf_a = z_a.reshape(B, N, 16)
    zf_b = z_b.reshape(B, N, 16)
    bi = np.arange(B)[:, None]
    z_flat_a = zf_a[bi, a_idx]               # [B,K,16]
    z_flat_b = zf_b[bi, b_idx]
    zg_a = z_a[bi, a_idx]                    # [B,K,4,4]
    zg_b = z_b[bi, b_idx]
    fg_a = fps_a[bi, a_idx]                  # [B,K,3]
    fg_b = fps_b[bi, b_idx]

    pd = fg_a[:, :, None, :] - fg_b[:, None, :, :]          # [B,K,K,3]
    zn_a = np.linalg.norm(z_flat_a, axis=-1)                # [B,K]
    zn_b = np.linalg.norm(z_flat_b, axis=-1)[:, None, :]    # [B,1,K]
    z_norm = np.maximum(zn_a[..., None], zn_b)              # [B,K,K]
    dist = np.linalg.norm(pd, axis=-1)
    scale = np.where(z_norm > 2.0 * dist, z_norm, 2.0 * dist)

    swap = zn_a[..., None] < zn_b                           # [B,K,K]
    pd = np.where(swap[..., None], -pd, pd)
    pz_a = np.broadcast_to(zg_a[:, :, None, :, :], (B, K, K, 4, 4))
    pz_b = np.broadcast_to(zg_b[:, None, :, :, :], (B, K, K, 4, 4))
    sw = swap[..., None, None]
    first = np.where(sw, pz_b, pz_a)
    second = np.where(sw, pz_a, pz_b)
    pz = np.concatenate([first, second], axis=-1)           # [B,K,K,4,8]

    # rotation frame (line2Rm), rows of R_inv are x, y, z
    z = pd / (np.linalg.norm(pd, axis=-1, keepdims=True) + EPS)
    ref = np.array([1.0, 0.0, 0.0], np.float32)
    x = ref - (z[..., 0:1]) * z
    x = x / (np.linalg.norm(x, axis=-1, keepdims=True) + EPS)
    y = np.cross(z, x)

    vec = pz[..., 1:, :]                                    # [B,K,K,3,8]
    rx = np.einsum('...j,...jc->...c', x, vec)
    ry = np.einsum('...j,...jc->...c', y, vec)
    rz = np.einsum('...j,...jc->...c', z, vec)
    pz_rot = np.concatenate(
        [pz[..., :1, :], rx[..., None, :], ry[..., None, :], rz[..., None, :]],
        axis=-2)                                            # [B,K,K,4,8]
    pd_rot2 = np.einsum('...j,...j->...', z, pd)            # z-component = dist

    inv_scale = (1.0 / scale).astype(np.float32)
    pz_rot = pz_rot * inv_scale[..., None, None]
    d_over = (pd_rot2 * inv_scale)[..., None]               # [B,K,K,1]

    p1 = _gelu_np(d_over @ pos_w1 + pos_b1)
    pos_feat = _gelu_np(p1 @ pos_w2 + pos_b2)               # [B,K,K,16]

    feat = np.concatenate(
        [pz_rot.reshape(B, K, K, 32), pos_feat], axis=-1).astype(np.float32)
    return feat, fg_a, fg_b, z_flat_a, z_flat_b


def kernel(**inputs):
    import ml_dtypes
    BF = ml_dtypes.bfloat16
    inp = {k: np.asarray(v) for k, v in inputs.items()}
    z_a = inp["z_a"].astype(np.float32)
    z_b = inp["z_b"].astype(np.float32)
    fps_a = inp["fps_a"].astype(np.float32)
    fps_b = inp["fps_b"].astype(np.float32)
    a_idx = inp["a_idx"].astype(np.int64)
    b_idx = inp["b_idx"].astype(np.int64)

    feat, fg_a, fg_b, z_flat_a, z_flat_b = _geometry(
        z_a, z_b, fps_a, fps_b, a_idx, b_idx,
        inp["pos_w1"].astype(np.float32), inp["pos_b1"].astype(np.float32),
        inp["pos_w2"].astype(np.float32), inp["pos_b2"].astype(np.float32))

    w1, w2, w3 = (inp["pw_w1"].astype(np.float32),
                  inp["pw_w2"].astype(np.float32),
                  inp["pw_w3"].astype(np.float32))
    b1, b2, b3 = (inp["pw_b1"].astype(np.float32),
                  inp["pw_b2"].astype(np.float32),
                  inp["pw_b3"].astype(np.float32))
    W1bd = np.zeros((96, 128), np.float32)
    W1bd[:48, :64] = w1
    W1bd[48:, 64:] = w1
    W2bd = np.zeros((128, 128), np.float32)
    W2bd[:64, :64] = w2
    W2bd[64:, 64:] = w2
    W3bd = np.zeros((128, 128), np.float32)
    W3bd[:64, :64] = w3
    W3bd[64:, 64:] = w3
    b1bd = np.concatenate([b1, b1]).reshape(128, 1).astype(np.float32)
    b2bd = np.concatenate([b2, b2]).reshape(128, 1).astype(np.float32)
    b3bd = np.concatenate([b3, b3]).reshape(128, 1).astype(np.float32)
    W1bd = W1bd.astype(BF)
    W2bd = W2bd.astype(BF)
    W3bd = W3bd.astype(BF)

    nc = _build_program()
    from concourse.bass_utils import run_bass_kernel_spmd

    in_maps = []
    for c in range(NCORES):
        fc = feat[c * BPC:(c + 1) * BPC].reshape(PAIRS, 48).astype(BF)
        # rows 0-47 = even pairs' features, 48-95 = odd pairs'
        ft = fc.reshape(NCOL, 2, 48).transpose(1, 2, 0).reshape(96, NCOL)
        in_maps.append({
            "featT": np.ascontiguousarray(ft),
            "w1bd": W1bd, "w2bd": W2bd, "w3bd": W3bd,
            "b1bd": b1bd, "b2bd": b2bd, "b3bd": b3bd,
        })
    _prog_cache["in_maps"] = in_maps
    res = run_bass_kernel_spmd(nc, in_maps, core_ids=list(range(NCORES)))

    out = np.empty((B, K, K, 102), np.float32)
    out[..., 0:3] = fg_a[:, :, None, :]
    out[..., 3:6] = fg_b[:, None, :, :]
    out[..., 6:22] = z_flat_a[:, :, None, :]
    out[..., 22:38] = z_flat_b[:, None, :, :]
    for c in range(NCORES):
        embT = np.asarray(res.results[c]["embT"]).astype(np.float32)
        # invert the 2-pair packing: [2,64,NCOL] -> [NCOL,2,64] -> [PAIRS,64]
        pairs = embT.reshape(2, 64, NCOL).transpose(2, 0, 1).reshape(PAIRS, 64)
        out[c * BPC:(c + 1) * BPC, ..., 38:102] = \
            pairs.reshape(BPC, K, K, 64)
    return out


def benchmark_device(n=4):
    """Re-run the cached device program; returns per-call walls (s)."""
    import time
    from concourse.bass_utils import run_bass_kernel_spmd
    nc = _prog_cache["nc"]
    in_maps = _prog_cache["in_maps"]
    walls = []
    for _ in range(n):
        t0 = time.time()
        run_bass_kernel_spmd(nc, in_maps, core_ids=list(range(NCORES)))
        walls.append(time.time() - t0)
    return walls


# revision 3
# speedup vs baseline: 1.3304x; 1.3304x over previous
"""Trainium2 kernel for nn_DSLRCollisionDecoder.

Data-parallel over batch B=256 across 8 NeuronCores (32 examples/core).
Device computes the dominant work: the pairwise 48->64->64->64 gelu MLP
with skip connection over B*K*K = 262144 pairs, packed 2 pairs/column
via block-diagonal weights so matmul/ACT run at full 128-partition width.
bf16 weights/activations/IO (fp32 PSUM accumulate) give 1-cycle/row
matmuls and half the DMA bytes; gelu runs on ScalarE over 2048-column
PSUM spans to amortize instruction overhead.
Host does index gathers, the small per-pair geometry (rotation frames),
and final channel concat.
"""
import sys
import numpy as np
from scipy.special import erf

sys.path.insert(0, "/opt/trn_rl_repo")

B, N, K = 256, 64, 32
EPS = 1e-8
NCORES = 8
BPC = B // NCORES          # batches per core
PAIRS = BPC * K * K        # 32768 pairs per core
NCOL = PAIRS // 2          # 16384 columns (2 pairs per column)
MTILE = 2048               # macro-tile columns (4 PSUM banks)
NMT = NCOL // MTILE        # 8 macro tiles
MM = 512                   # matmul free dim (1 PSUM bank)

_prog_cache = {}


def _gelu_np(x):
    return 0.5 * x * (1.0 + erf(x / np.sqrt(2.0).astype(np.float32)))


def _build_program():
    if "nc" in _prog_cache:
        return _prog_cache["nc"]
    import concourse.bacc as bacc
    import concourse.tile as tile
    from concourse import mybir
    from concourse.alu_op_type import AluOpType
    from bass_rust import ActivationFunctionType as AF

    F32 = mybir.dt.float32
    BF16 = mybir.dt.bfloat16
    nc = bacc.Bacc("TRN2", target_bir_lowering=False, debug=False,
                   num_devices=NCORES)
    ft_d = nc.declare_dram_parameter("featT", [96, NCOL], BF16, isOutput=False)
    w1_d = nc.declare_dram_parameter("w1bd", [96, 128], BF16, isOutput=False)
    w2_d = nc.declare_dram_parameter("w2bd", [128, 128], BF16, isOutput=False)
    w3_d = nc.declare_dram_parameter("w3bd", [128, 128], BF16, isOutput=False)
    b1_d = nc.declare_dram_parameter("b1bd", [128, 1], F32, isOutput=False)
    b2_d = nc.declare_dram_parameter("b2bd", [128, 1], F32, isOutput=False)
    b3_d = nc.declare_dram_parameter("b3bd", [128, 1], F32, isOutput=False)
    out_d = nc.declare_dram_parameter("embT", [128, NCOL], BF16, isOutput=True)

    with tile.TileContext(nc) as tc:
        with (
            tc.tile_pool(name="w", bufs=1) as wp,
            tc.tile_pool(name="xfull", bufs=1) as xf,
            tc.tile_pool(name="io", bufs=3) as iop,
            tc.tile_pool(name="act", bufs=2) as ac,
            tc.tile_pool(name="ps", bufs=2, space="PSUM") as pp,
        ):
            tw1 = wp.tile([96, 128], BF16, tag="w1")
            tw2 = wp.tile([128, 128], BF16, tag="w2")
            tw3 = wp.tile([128, 128], BF16, tag="w3")
            tb1 = wp.tile([128, 1], F32, tag="b1")
            tb2 = wp.tile([128, 1], F32, tag="b2")
            tb3 = wp.tile([128, 1], F32, tag="b3")
            nc.sync.dma_start(tw1[:], w1_d[:, :])
            nc.sync.dma_start(tw2[:], w2_d[:, :])
            nc.sync.dma_start(tw3[:], w3_d[:, :])
            nc.sync.dma_start(tb1[:], b1_d[:, :])
            nc.sync.dma_start(tb2[:], b2_d[:, :])
            nc.sync.dma_start(tb3[:], b3_d[:, :])
            # Layer-major order: within each phase the activation of tile i
            # overlaps the matmuls of tile i+1 (PSUM ping-pong), so ScalarE
            # (the bottleneck) streams without stalls and the PE stream is
            # dense enough to release the HAM clock gate.
            x1 = xf.tile([128, NCOL], BF16, tag="x1")
            x2 = xf.tile([128, NCOL], BF16, tag="x2")
            for i in range(NMT):
                sl = slice(i * MTILE, (i + 1) * MTILE)
                ft = iop.tile([96, MTILE], BF16, tag="ft")
                nc.sync.dma_start(ft[:], ft_d[:, sl])
                ps = pp.tile([128, MTILE], F32, tag="ps")
                for j in range(MTILE // MM):
                    jm = slice(j * MM, (j + 1) * MM)
                    nc.tensor.matmul(ps[:, jm], tw1[:], ft[:, jm],
                                     start=True, stop=True)
                nc.scalar.activation(x1[:, sl], ps[:], AF.Gelu, bias=tb1[:, :])
            for i in range(NMT):
                sl = slice(i * MTILE, (i + 1) * MTILE)
                ps = pp.tile([128, MTILE], F32, tag="ps")
                for j in range(MTILE // MM):
                    jm = slice(i * MTILE + j * MM, i * MTILE + (j + 1) * MM)
                    nc.tensor.matmul(ps[:, j * MM:(j + 1) * MM], tw2[:],
                                     x1[:, jm], start=True, stop=True)
                nc.scalar.activation(x2[:, sl], ps[:], AF.Gelu, bias=tb2[:, :])
            for i in range(NMT):
                sl = slice(i * MTILE, (i + 1) * MTILE)
                ps = pp.tile([128, MTILE], F32, tag="ps")
                for j in range(MTILE // MM):
                    jm = slice(i * MTILE + j * MM, i * MTILE + (j + 1) * MM)
                    nc.tensor.matmul(ps[:, j * MM:(j + 1) * MM], tw3[:],
                                     x2[:, jm], start=True, stop=True)
                x3 = ac.tile([128, MTILE], BF16, tag="x3")
                nc.scalar.activation(x3[:], ps[:], AF.Gelu, bias=tb3[:, :])
                emb = ac.tile([128, MTILE], BF16, tag="emb")
                nc.vector.tensor_tensor(emb[:], x3[:], x1[:, sl],
                                        op=AluOpType.add)
                nc.sync.dma_start(out_d[:, sl], emb[:])
    nc.compile()
    _prog_cache["nc"] = nc
    return nc


def _geometry(z_a, z_b, fps_a, fps_b, a_idx, b_idx,
              pos_w1, pos_b1, pos_w2, pos_b2):
    """Gathers + per-pair frame/rotation/pos-MLP; returns feat + concat parts."""
    zf_a = z_a.reshape(B, N, 16)
    zf_b = z_b.reshape(B, N, 16)
    bi = np.arange(B)[:, None]
    z_flat_a = zf_a[bi, a_idx]               # [B,K,16]
    z_flat_b = zf_b[bi, b_idx]
    zg_a = z_a[bi, a_idx]                    # [B,K,4,4]
    zg_b = z_b[bi, b_idx]
    fg_a = fps_a[bi, a_idx]                  # [B,K,3]
    fg_b = fps_b[bi, b_idx]

    pd = fg_a[:, :, None, :] - fg_b[:, None, :, :]          # [B,K,K,3]
    zn_a = np.linalg.norm(z_flat_a, axis=-1)                # [B,K]
    zn_b = np.linalg.norm(z_flat_b, axis=-1)[:, None, :]    # [B,1,K]
    z_norm = np.maximum(zn_a[..., None], zn_b)              # [B,K,K]
    dist = np.linalg.norm(pd, axis=-1)
    scale = np.where(z_norm > 2.0 * dist, z_norm, 2.0 * dist)

    swap = zn_a[..., None] < zn_b                           # [B,K,K]
    pd = np.where(swap[..., None], -pd, pd)
    pz_a = np.broadcast_to(zg_a[:, :, None, :, :], (B, K, K, 4, 4))
    pz_b = np.broadcast_to(zg_b[:, None, :, :, :], (B, K, K, 4, 4))
    sw = swap[..., None, None]
    first = np.where(sw, pz_b, pz_a)
    second = np.where(sw, pz_a, pz_b)
    pz = np.concatenate([first, second], axis=-1)           # [B,K,K,4,8]

    # rotation frame (line2Rm), rows of R_inv are x, y, z
    z = pd / (np.linalg.norm(pd, axis=-1, keepdims=True) + EPS)
    ref = np.array([1.0, 0.0, 0.0], np.float32)
    x = ref - (z[..., 0:1]) * z
    x = x / (np.linalg.norm(x, axis=-1, keepdims=True) + EPS)
    y = np.cross(z, x)

    vec = pz[..., 1:, :]                                    # [B,K,K,3,8]
    rx = np.einsum('...j,...jc->...c', x, vec)
    ry = np.einsum('...j,...jc->...c', y, vec)
    rz = np.einsum('...j,...jc->...c', z, vec)
    pz_rot = np.concatenate(
        [pz[..., :1, :], rx[..., None, :], ry[..., None, :], rz[..., None, :]],
        axis=-2)                                            # [B,K,K,4,8]
    pd_rot2 = np.einsum('...j,...j->...', z, pd)            # z-component = dist

    inv_scale = (1.0 / scale).astype(np.float32)
    pz_rot = pz_rot * inv_scale[..., None, None]
    d_over = (pd_rot2 * inv_scale)[..., None]               # [B,K,K,1]

    p1 = _gelu_np(d_over @ pos_w1 + pos_b1)
    pos_feat = _gelu_np(p1 @ pos_w2 + pos_b2)               # [B,K,K,16]

    feat = np.concatenate(
        [pz_rot.reshape(B, K, K, 32), pos_feat], axis=-1).astype(np.float32)
    return feat, fg_a, fg_b, z_flat_a, z_flat_b


def kernel(**inputs):
    import ml_dtypes
    BF = ml_dtypes.bfloat16
    inp = {k: np.asarray(v) for k, v in inputs.items()}
    z_a = inp["z_a"].astype(np.float32)
    z_b = inp["z_b"].astype(np.float32)
    fps_a = inp["fps_a"].astype(np.float32)
    fps_b = inp["fps_b"].astype(np.float32)
    a_idx = inp["a_idx"].astype(np.int64)
    b_idx = inp["b_idx"].astype(np.int64)

    feat, fg_a, fg_b, z_flat_a, z_flat_b = _geometry(
        z_a, z_b, fps_a, fps_b, a_idx, b_idx,
        inp["pos_w1"].astype(np.float32), inp["pos_b1"].astype(np.float32),
        inp["pos_w2"].astype(np.float32), inp["pos_b2"].astype(np.float32))

    w1, w2, w3 = (inp["pw_w1"].astype(np.float32),
                  inp["pw_w2"].astype(np.float32),
                  inp["pw_w3"].astype(np.float32))
    b1, b2, b3 = (inp["pw_b1"].astype(np.float32),
                  inp["pw_b2"].astype(np.float32),
                  inp["pw_b3"].astype(np.float32))
    W1bd = np.zeros((96, 128), np.float32)
    W1bd[:48, :64] = w1
    W1bd[48:, 64:] = w1
    W2bd = np.zeros((128, 128), np.float32)
    W2bd[:64, :64] = w2
    W2bd[64:, 64:] = w2
    W3bd = np.zeros((128, 128), np.float32)
    W3bd[:64, :64] = w3
    W3bd[64:, 64:] = w3
    b1bd = np.concatenate([b1, b1]).reshape(128, 1).astype(np.float32)
    b2bd = np.concatenate([b2, b2]).reshape(128, 1).astype(np.float32)
    b3bd = np.concatenate([b3, b3]).reshape(128, 1).astype(np.float32)
    W1bd = W1bd.astype(BF)
    W2bd = W2bd.astype(BF)
    W3bd = W3bd.astype(BF)

    nc = _build_program()
    from concourse.bass_utils import run_bass_kernel_spmd

    in_maps = []
    for c in range(NCORES):
        fc = feat[c * BPC:(c + 1) * BPC].reshape(PAIRS, 48).astype(BF)
        # rows 0-47 = even pairs' features, 48-95 = odd pairs'
        ft = fc.reshape(NCOL, 2, 48).transpose(1, 2, 0).reshape(96, NCOL)
        in_maps.append({
            "featT": np.ascontiguousarray(ft),
            "w1bd": W1bd, "w2bd": W2bd, "w3bd": W3bd,
            "b1bd": b1bd, "b2bd": b2bd, "b3bd": b3bd,
        })
    _prog_cache["in_maps"] = in_maps
    res = run_bass_kernel_spmd(nc, in_maps, core_ids=list(range(NCORES)))

    out = np.empty((B, K, K, 102), np.float32)
    out[..., 0:3] = fg_a[:, :, None, :]
    out[..., 3:6] = fg_b[:, None, :, :]
    out[..., 6:22] = z_flat_a[:, :, None, :]
    out[..., 22:38] = z_flat_b[:, None, :, :]
    for c in range(NCORES):
        embT = np.asarray(res.results[c]["embT"]).astype(np.float32)
        # invert the 2-pair packing: [2,64,NCOL] -> [NCOL,2,64] -> [PAIRS,64]
        pairs = embT.reshape(2, 64, NCOL).transpose(2, 0, 1).reshape(PAIRS, 64)
        out[c * BPC:(c + 1) * BPC, ..., 38:102] = \
            pairs.reshape(BPC, K, K, 64)
    return out


def benchmark_device(n=4):
    """Re-run the cached device program; returns per-call walls (s)."""
    import time
    from concourse.bass_utils import run_bass_kernel_spmd
    nc = _prog_cache["nc"]
    in_maps = _prog_cache["in_maps"]
    walls = []
    for _ in range(n):
        t0 = time.time()
        run_bass_kernel_spmd(nc, in_maps, core_ids=list(range(NCORES)))
        walls.append(time.time() - t0)
    return walls


# revision 6
# speedup vs baseline: 1.8032x; 1.3555x over previous
"""Trainium2 kernel for nn_DSLRCollisionDecoder.

Data-parallel over batch B=256 across 8 NeuronCores. Device computes the
dominant work: the pairwise 48->64->64->64 gelu MLP with skip connection,
packed 2 pairs/column via block-diagonal weights so matmul/ACT run at
full 128-partition width.

Key optimizations:
- a_idx/b_idx are sampled with replacement, so only ~63% of the K*K
  pairs per example are distinct: the device evaluates each distinct
  (a_val, b_val) pair once; the host expands results back. Examples are
  LPT-balanced across cores by distinct-pair count.
- bf16 weights/activations/IO (fp32 PSUM accumulate): 1-cycle/row
  matmuls, half the DMA bytes.
- Layer-major loop: ScalarE (gelu, the bottleneck) streams without
  stalls while the PE fills the next PSUM tile (ping-pong).
- gelu over 2048-column PSUM spans amortizes ACT instruction overhead.
- DMA issues spread across idle engine queues; small final tile
  shortens the kernel tail.

Host does index gathers, the small per-pair geometry (rotation frames),
the pos-MLP, and the final channel concat.
"""
import sys
import numpy as np
from scipy.special import erf

sys.path.insert(0, "/opt/trn_rl_repo")

B, N, K = 256, 64, 32
EPS = 1e-8
NCORES = 8
MM = 512                     # matmul free dim (1 PSUM bank)

NCOL2 = 10752                # deduped: 5x2048 + 1x512 columns
BUDGET = NCOL2 * 2           # 21504 pairs per core (balanced max ~20550)
TILES = [2048] * 5 + [512]

NCOL_FULL = 16384            # fallback: all 32768 pairs per core
TILES_FULL = [2048] * 8

_prog_cache = {}


def _gelu_np(x):
    return 0.5 * x * (1.0 + erf(x / np.sqrt(2.0).astype(np.float32)))


def _build_program(ncol, tiles):
    key = "nc_%d" % ncol
    if key in _prog_cache:
        return _prog_cache[key]
    import concourse.bacc as bacc
    import concourse.tile as tile
    from concourse import mybir
    from concourse.alu_op_type import AluOpType
    from bass_rust import ActivationFunctionType as AF

    F32 = mybir.dt.float32
    BF16 = mybir.dt.bfloat16
    nc = bacc.Bacc("TRN2", target_bir_lowering=False, debug=False,
                   num_devices=NCORES)
    ft_d = nc.declare_dram_parameter("featT", [96, ncol], BF16, isOutput=False)
    wp_d = nc.declare_dram_parameter("wpack", [128, 384], BF16, isOutput=False)
    bp_d = nc.declare_dram_parameter("bpack", [128, 3], F32, isOutput=False)
    out_d = nc.declare_dram_parameter("embT", [128, ncol], BF16, isOutput=True)

    starts = np.cumsum([0] + tiles)[:-1]

    with tile.TileContext(nc) as tc:
        with (
            tc.tile_pool(name="w", bufs=1) as wp,
            tc.tile_pool(name="xfull", bufs=1) as xf,
            tc.tile_pool(name="io", bufs=3) as iop,
            tc.tile_pool(name="act", bufs=2) as ac,
            tc.tile_pool(name="ps", bufs=2, space="PSUM") as pp,
        ):
            # Weights/biases in 2 DMAs on the (otherwise idle) gpsimd
            # queue so feature DMAs own the sync/vector queues.
            twp = wp.tile([128, 384], BF16, tag="wpack")
            tbp = wp.tile([128, 3], F32, tag="bpack")
            nc.gpsimd.dma_start(twp[:], wp_d[:, :])
            nc.gpsimd.dma_start(tbp[:], bp_d[:, :])
            tw = [twp[0:96, 0:128], twp[:, 128:256], twp[:, 256:384]]
            tb = [tbp[:, 0:1], tbp[:, 1:2], tbp[:, 2:3]]

            x1 = xf.tile([128, ncol], BF16, tag="x1")
            x2 = xf.tile([128, ncol], BF16, tag="x2")
            # Layer-major: activation of tile i overlaps matmuls of tile
            # i+1 (PSUM ping-pong) so ScalarE never stalls.
            for i, (t0, tww) in enumerate(zip(starts, tiles)):
                ft = iop.tile([96, tww], BF16, tag="ft")
                for c0 in range(0, tww, 1024):
                    cw = min(1024, tww - c0)
                    eng = nc.sync if (c0 // 1024) % 2 == 0 else nc.gpsimd
                    eng.dma_start(ft[:, c0:c0 + cw],
                                  ft_d[:, t0 + c0:t0 + c0 + cw])
                ps = pp.tile([128, tww], F32, tag="ps")
                for j in range(tww // MM):
                    jm = slice(j * MM, (j + 1) * MM)
                    nc.tensor.matmul(ps[:, jm], tw[0], ft[:, jm],
                                     start=True, stop=True)
                nc.scalar.activation(x1[:, t0:t0 + tww], ps[:], AF.Gelu,
                                     bias=tb[0])
            for i, (t0, tww) in enumerate(zip(starts, tiles)):
                ps = pp.tile([128, tww], F32, tag="ps")
                for j in range(tww // MM):
                    jm = slice(t0 + j * MM, t0 + (j + 1) * MM)
                    nc.tensor.matmul(ps[:, j * MM:(j + 1) * MM], tw[1],
                                     x1[:, jm], start=True, stop=True)
                nc.scalar.activation(x2[:, t0:t0 + tww], ps[:], AF.Gelu,
                                     bias=tb[1])
            for i, (t0, tww) in enumerate(zip(starts, tiles)):
                ps = pp.tile([128, tww], F32, tag="ps")
                for j in range(tww // MM):
                    jm = slice(t0 + j * MM, t0 + (j + 1) * MM)
                    nc.tensor.matmul(ps[:, j * MM:(j + 1) * MM], tw[2],
                                     x2[:, jm], start=True, stop=True)
                x3 = ac.tile([128, tww], BF16, tag="x3")
                nc.scalar.activation(x3[:], ps[:], AF.Gelu, bias=tb[2])
                emb = ac.tile([128, tww], BF16, tag="emb")
                nc.vector.tensor_tensor(emb[:], x3[:], x1[:, t0:t0 + tww],
                                        op=AluOpType.add)
                nc.gpsimd.dma_start(out_d[:, t0:t0 + tww], emb[:])
    nc.compile()
    _prog_cache[key] = nc
    return nc


def _geometry(z_a, z_b, fps_a, fps_b, a_idx, b_idx,
              pos_w1, pos_b1, pos_w2, pos_b2):
    """Gathers + per-pair frame/rotation/pos-MLP; returns feat + concat parts."""
    zf_a = z_a.reshape(B, N, 16)
    zf_b = z_b.reshape(B, N, 16)
    bi = np.arange(B)[:, None]
    z_flat_a = zf_a[bi, a_idx]               # [B,K,16]
    z_flat_b = zf_b[bi, b_idx]
    zg_a = z_a[bi, a_idx]                    # [B,K,4,4]
    zg_b = z_b[bi, b_idx]
    fg_a = fps_a[bi, a_idx]                  # [B,K,3]
    fg_b = fps_b[bi, b_idx]

    pd = fg_a[:, :, None, :] - fg_b[:, None, :, :]          # [B,K,K,3]
    zn_a = np.linalg.norm(z_flat_a, axis=-1)                # [B,K]
    zn_b = np.linalg.norm(z_flat_b, axis=-1)[:, None, :]    # [B,1,K]
    z_norm = np.maximum(zn_a[..., None], zn_b)              # [B,K,K]
    dist = np.linalg.norm(pd, axis=-1)
    scale = np.where(z_norm > 2.0 * dist, z_norm, 2.0 * dist)

    swap = zn_a[..., None] < zn_b                           # [B,K,K]
    pd = np.where(swap[..., None], -pd, pd)
    pz_a = np.broadcast_to(zg_a[:, :, None, :, :], (B, K, K, 4, 4))
    pz_b = np.broadcast_to(zg_b[:, None, :, :, :], (B, K, K, 4, 4))
    sw = swap[..., None, None]
    first = np.where(sw, pz_b, pz_a)
    second = np.where(sw, pz_a, pz_b)
    pz = np.concatenate([first, second], axis=-1)           # [B,K,K,4,8]

    # rotation frame (line2Rm), rows of R_inv are x, y, z
    z = pd / (np.linalg.norm(pd, axis=-1, keepdims=True) + EPS)
    ref = np.array([1.0, 0.0, 0.0], np.float32)
    x = ref - (z[..., 0:1]) * z
    x = x / (np.linalg.norm(x, axis=-1, keepdims=True) + EPS)
    y = np.cross(z, x)

    vec = pz[..., 1:, :]                                    # [B,K,K,3,8]
    rx = np.einsum('...j,...jc->...c', x, vec)
    ry = np.einsum('...j,...jc->...c', y, vec)
    rz = np.einsum('...j,...jc->...c', z, vec)
    pz_rot = np.concatenate(
        [pz[..., :1, :], rx[..., None, :], ry[..., None, :], rz[..., None, :]],
        axis=-2)                                            # [B,K,K,4,8]
    pd_rot2 = np.einsum('...j,...j->...', z, pd)            # z-component = dist

    inv_scale = (1.0 / scale).astype(np.float32)
    pz_rot = pz_rot * inv_scale[..., None, None]
    d_over = (pd_rot2 * inv_scale)[..., None]               # [B,K,K,1]

    p1 = _gelu_np(d_over @ pos_w1 + pos_b1)
    pos_feat = _gelu_np(p1 @ pos_w2 + pos_b2)               # [B,K,K,16]

    feat = np.concatenate(
        [pz_rot.reshape(B, K, K, 32), pos_feat], axis=-1).astype(np.float32)
    return feat, fg_a, fg_b, z_flat_a, z_flat_b


def _dedup_plan(a_idx, b_idx):
    """Per-example distinct-pair plan + LPT assignment of examples to cores.

    Returns (plans, core_examples, overflow) where plans[ex] =
    (rep_a, rep_b, inv_a, inv_b, n_pairs): rep_* are representative
    positions of the distinct index values, inv_* map each original
    position to its representative's rank.
    """
    plans = []
    for ex in range(B):
        ua, ra = np.unique(a_idx[ex], return_index=True)
        ub, rb = np.unique(b_idx[ex], return_index=True)
        inv_a = np.searchsorted(ua, a_idx[ex])
        inv_b = np.searchsorted(ub, b_idx[ex])
        plans.append((ra, rb, inv_a, inv_b, len(ua) * len(ub)))
    order = sorted(range(B), key=lambda ex: -plans[ex][4])
    loads = [0] * NCORES
    core_examples = [[] for _ in range(NCORES)]
    for ex in order:
        c = loads.index(min(loads))
        core_examples[c].append(ex)
        loads[c] += plans[ex][4]
    return plans, core_examples, max(loads) > BUDGET


def kernel(**inputs):
    import ml_dtypes
    BF = ml_dtypes.bfloat16
    inp = {k: np.asarray(v) for k, v in inputs.items()}
    z_a = inp["z_a"].astype(np.float32)
    z_b = inp["z_b"].astype(np.float32)
    fps_a = inp["fps_a"].astype(np.float32)
    fps_b = inp["fps_b"].astype(np.float32)
    a_idx = inp["a_idx"].astype(np.int64)
    b_idx = inp["b_idx"].astype(np.int64)

    feat, fg_a, fg_b, z_flat_a, z_flat_b = _geometry(
        z_a, z_b, fps_a, fps_b, a_idx, b_idx,
        inp["pos_w1"].astype(np.float32), inp["pos_b1"].astype(np.float32),
        inp["pos_w2"].astype(np.float32), inp["pos_b2"].astype(np.float32))

    w1, w2, w3 = (inp["pw_w1"].astype(np.float32),
                  inp["pw_w2"].astype(np.float32),
                  inp["pw_w3"].astype(np.float32))
    b1, b2, b3 = (inp["pw_b1"].astype(np.float32),
                  inp["pw_b2"].astype(np.float32),
                  inp["pw_b3"].astype(np.float32))
    Wpack = np.zeros((128, 384), np.float32)
    Wpack[:48, 0:64] = w1
    Wpack[48:96, 64:128] = w1
    Wpack[:64, 128:192] = w2
    Wpack[64:, 192:256] = w2
    Wpack[:64, 256:320] = w3
    Wpack[64:, 320:384] = w3
    Wpack = Wpack.astype(BF)
    Bpack = np.stack([np.concatenate([b1, b1]),
                      np.concatenate([b2, b2]),
                      np.concatenate([b3, b3])], axis=1).astype(np.float32)

    plans, core_examples, overflow = _dedup_plan(a_idx, b_idx)
    if overflow:
        # pathological index distribution: fall back to all pairs,
        # contiguous example blocks
        ncol, tiles = NCOL_FULL, TILES_FULL
        core_examples = [list(range(c * (B // NCORES),
                                    (c + 1) * (B // NCORES)))
                         for c in range(NCORES)]
        plans = [(np.arange(K), np.arange(K), np.arange(K), np.arange(K),
                  K * K) for _ in range(B)]
    else:
        ncol, tiles = NCOL2, TILES

    nc = _build_program(ncol, tiles)
    from concourse.bass_utils import run_bass_kernel_spmd

    in_maps = []
    for c in range(NCORES):
        fc = np.zeros((2 * ncol, 48), np.float32)
        o = 0
        for ex in core_examples[c]:
            ra, rb, _, _, npair = plans[ex]
            fc[o:o + npair] = feat[ex][np.ix_(ra, rb)].reshape(npair, 48)
            o += npair
        fcb = fc.astype(BF)
        # rows 0-47 = even pairs' features, 48-95 = odd pairs'
        ft = fcb.reshape(ncol, 2, 48).transpose(1, 2, 0).reshape(96, ncol)
        in_maps.append({
            "featT": np.ascontiguousarray(ft),
            "wpack": Wpack, "bpack": Bpack,
        })
    _prog_cache["in_maps"] = in_maps
    _prog_cache["nc"] = nc
    res = run_bass_kernel_spmd(nc, in_maps, core_ids=list(range(NCORES)))

    out = np.empty((B, K, K, 102), np.float32)
    out[..., 0:3] = fg_a[:, :, None, :]
    out[..., 3:6] = fg_b[:, None, :, :]
    out[..., 6:22] = z_flat_a[:, :, None, :]
    out[..., 22:38] = z_flat_b[:, None, :, :]
    for c in range(NCORES):
        embT = np.asarray(res.results[c]["embT"]).astype(np.float32)
        # invert the 2-pair packing: [2,64,ncol] -> [ncol,2,64] -> pairs
        pairs = embT.reshape(2, 64, ncol).transpose(2, 0, 1).reshape(2 * ncol, 64)
        o = 0
        for ex in core_examples[c]:
            ra, rb, inv_a, inv_b, npair = plans[ex]
            da, db = len(ra), len(rb)
            blk = pairs[o:o + npair].reshape(da, db, 64)
            out[ex, ..., 38:102] = blk[inv_a][:, inv_b]
            o += npair
    return out


def benchmark_device(n=4):
    """Re-run the cached device program; returns per-call walls (s)."""
    import time
    from concourse.bass_utils import run_bass_kernel_spmd
    nc = _prog_cache["nc"]
    in_maps = _prog_cache["in_maps"]
    walls = []
    for _ in range(n):
        t0 = time.time()
        run_bass_kernel_spmd(nc, in_maps, core_ids=list(range(NCORES)))
        walls.append(time.time() - t0)
    return walls
